# revision 1
# baseline (speedup 1.0000x reference)
"""Trainium2 Bass kernel for nn_AutoregressiveConvLSTM.

Data-parallel over batch: 32 images -> 8 cores x 4 images.

Layout per core: every 2D field (x channel, zi channel, h, c, gates) is stored
as (partition = H row 0..127, free = img*130 + 1 + w) with zero pad columns at
w offsets 0 and 129 of each image so that the 3 horizontal conv taps are plain
free-dim offset reads (dx in 0..2).

3x3 SAME convs run on the TensorEngine as banded matmuls: out = B.T @ rhs with
B[h, h'] = W[h-h'+1, dx, ci, co] a tridiagonal 128x128 "band" (vertical taps),
one matmul per (dx, ci) accumulating in PSUM; rhs is the plane with the
free-dim offset dx.

Recurrence avoids ACT table switches by using only tanh + exp
(exp_and_others set):  sigmoid(v) = 0.5*(tanh(v/2)+1).  h is stored doubled
(h2 = 2h = (tanh(o/2)+1)*tanh(c)) and the 0.5 is folded into the Whh/Wout
bands.

log prob: z = ((mu+b0) - x) * exp(-ls-b1); per-image Sum z^2 via ACT Square
with accum_out; Sum ls via DVE tensor_reduce; final cross-partition reduction
via a ones-vector matmul.
"""

import sys
import numpy as np

for _p in ("/opt/trn_rl_repo", "/root/.axon_site/_ro/trn_rl_repo"):
    if _p not in sys.path:
        sys.path.insert(0, _p)

import concourse.bacc as bacc
import concourse.mybir as mybir
from concourse import bass, tile
from concourse.bass_utils import run_bass_kernel_spmd

F32 = mybir.dt.float32
F32R = mybir.dt.float32r
AF = mybir.ActivationFunctionType
ALU = mybir.AluOpType

B, C, H, W = 32, 16, 128, 128
NCORES = 8
BL = B // NCORES          # images per core = 4
WP = W + 2                # padded row width = 130
LOG2PI = 1.8378770664093453

# band tensor indexing
N_IN = 3                        # conv_in: dx
N_IH = 8 * 3                    # conv_ih: co, dx
N_HH = 8 * 2 * 3                # conv_hh: co, ci, dx
N_OUT = 2 * 2 * 3               # conv_out: co, ci, dx
NBANDS = N_IN + N_IH + N_HH + N_OUT   # 87


def _band(w_col):
    """Build the 128x128 tridiagonal lhsT for one (ky tap column) of a 3-tap
    vertical conv: lhsT[h, h'] = w_col[h - h' + 1] for |h-h'| <= 1."""
    Bm = np.zeros((H, H), np.float32)
    idx = np.arange(H)
    for ky in range(3):
        hh = idx + ky - 1          # input row feeding output row idx
        m = (hh >= 0) & (hh < H)
        Bm[hh[m], idx[m]] = w_col[ky]
    return Bm


def _build_bands(Win, Wih, Whh, Wout):
    """All band matrices as one (87,128,128) array (lhsT layout)."""
    bands = np.zeros((NBANDS, H, H), np.float32)
    k = 0
    for dx in range(3):                        # conv_in (1->1)
        bands[k] = _band(Win[:, dx, 0, 0]); k += 1
    for co in range(8):                        # conv_ih (1->8)
        for dx in range(3):
            bands[k] = _band(Wih[:, dx, 0, co]); k += 1
    for co in range(8):                        # conv_hh (2->8), x0.5 (h2)
        for ci in range(2):
            for dx in range(3):
                bands[k] = _band(0.5 * Whh[:, dx, ci, co]); k += 1
    Wout_y = Wout[:, :, :2, :]                 # cond features are zero
    for co in range(2):                        # conv_out (2->2), x0.5 (h2)
        for ci in range(2):
            for dx in range(3):
                bands[k] = _band(0.5 * Wout_y[:, dx, ci, co]); k += 1
    assert k == NBANDS
    return bands


_CACHED = None


def _build_program(nsteps=None, skip_rec=False):
    import os
    if nsteps is None:
        nsteps = int(os.environ.get("KERNEL_T", C - 1))
    nc = bacc.Bacc(None, target_bir_lowering=False)

    xp_d = nc.dram_tensor("xp", [H, C * BL * WP], F32, kind="ExternalInput")
    bands_d = nc.dram_tensor("bands", [H, NBANDS * H], F32R, kind="ExternalInput")
    cols_d = nc.dram_tensor("cols", [H, 16], F32, kind="ExternalInput")
    out_d = nc.dram_tensor("out", [BL, 1], F32, kind="ExternalOutput")

    T = C - 1  # 15 recurrence steps
    TR = nsteps

    with tile.TileContext(nc) as tc:
        with (
            tc.tile_pool(name="const", bufs=1) as cpool,
            tc.tile_pool(name="state", bufs=1) as spool,
            tc.tile_pool(name="work", bufs=2) as wpool,
            tc.tile_pool(name="once", bufs=1) as opool,
            tc.tile_pool(name="psum", bufs=4, space=bass.MemorySpace.PSUM) as ppool,
        ):
            xall = cpool.tile([H, C, BL, WP], F32, tag="xall")
            bandsb = cpool.tile([H, NBANDS, H], F32R, tag="bands")
            cols = cpool.tile([H, 16], F32, tag="cols")
            ziall = cpool.tile([H, T, BL, WP], F32R, tag="ziall")
            ones = cpool.tile([H, 1], F32, tag="ones")

            hpair = spool.tile([H, 2, BL, WP], F32R, tag="hpair")
            cst = spool.tile([H, 2, BL, W], F32, tag="cst")
            sqcols = spool.tile([H, BL, C], F32, tag="sqcols")
            lscols = spool.tile([H, BL, C], F32, tag="lscols")

            # ---- load inputs ----
            nc.sync.dma_start(xall[:], xp_d[:])
            nc.sync.dma_start(bandsb[:], bands_d[:])
            nc.sync.dma_start(cols[:], cols_d[:])

            nc.gpsimd.memset(ziall[:].bitcast(F32), 0.0)
            nc.gpsimd.memset(hpair[:].bitcast(F32), 0.0)
            nc.gpsimd.memset(cst[:], 0.0)
            nc.gpsimd.memset(sqcols[:], 0.0)
            nc.gpsimd.memset(lscols[:], 0.0)
            nc.gpsimd.memset(ones[:], 1.0)

            def band(i):
                return bandsb[:, i, :]

            # ---- precompute zi_t = conv_in(x_t) + b_in for t in 0..14 ----
            for t in range(T):
                xr = wpool.tile([H, BL, WP], F32R, tag="xr")
                nc.vector.tensor_copy(xr[:], xall[:, t])
                zps = ppool.tile([H, BL, W], F32, tag="ps")
                for dx in range(3):
                    nc.tensor.matmul(
                        zps[:], band(dx), xr[:, :, dx:dx + W],
                        start=(dx == 0), stop=(dx == 2),
                    )
                # zi -> SBUF padded cols, +b_in
                nc.scalar.activation(
                    ziall[:, t, :, 1:1 + W], zps[:], AF.Identity,
                    bias=cols[:, 0:1],
                )

            # ---- channel 0 logprob: z0 = (x0 - b0) * exp(-b1) ----
            # Square(scale*x + bias) with scale=e^{-b1}, bias=-b0*e^{-b1}
            zjunk = opool.tile([H, BL, W], F32, tag="zjunk")
            for im in range(BL):
                nc.scalar.activation(
                    zjunk[:, im, :], xall[:, 0, im, 1:1 + W], AF.Square,
                    scale=cols[:, 2:3], bias=cols[:, 3:4],
                    accum_out=sqcols[:, im, C - 1:C],
                )

            # ---- recurrence ----
            for t in range(0 if skip_rec else TR):
                th = []  # tanh-gate tiles: i,g,f,o
                for g in range(4):
                    gps = ppool.tile([H, 2, BL, W], F32, tag="ps")
                    for half in range(2):
                        co = g * 2 + half
                        mms = []
                        for dx in range(3):
                            mms.append((N_IN + co * 3 + dx,
                                        ziall[:, t, :, dx:dx + W]))
                        if t > 0:
                            for ci in range(2):
                                for dx in range(3):
                                    mms.append((
                                        N_IN + N_IH + (co * 2 + ci) * 3 + dx,
                                        hpair[:, ci, :, dx:dx + W]))
                        for k, (bi, rhs) in enumerate(mms):
                            nc.tensor.matmul(
                                gps[:, half], band(bi), rhs,
                                start=(k == 0), stop=(k == len(mms) - 1),
                            )
                    tg = wpool.tile([H, 2, BL, W], F32, tag=f"th{g}")
                    # i,f,o: tanh(v/2 + bias'); g: tanh(v + bias)
                    scale = 1.0 if g == 1 else 0.5
                    for half in range(2):
                        co = g * 2 + half
                        nc.scalar.activation(
                            tg[:, half], gps[:, half], AF.Tanh,
                            scale=scale, bias=cols[:, 5 + co:6 + co],
                        )
                    th.append(tg)
                ti, tgg, tf, to = th

                u1 = wpool.tile([H, 2, BL, W], F32, tag="u1")
                u2 = wpool.tile([H, 2, BL, W], F32, tag="u2")
                nc.vector.scalar_tensor_tensor(
                    u1[:], tf[:], 1.0, cst[:], ALU.add, ALU.mult)
                nc.vector.scalar_tensor_tensor(
                    u2[:], ti[:], 1.0, tgg[:], ALU.add, ALU.mult)
                s2 = opool.tile([H, 2, BL, W], F32, tag="s2")
                nc.vector.tensor_add(s2[:], u1[:], u2[:])        # s2 = 2*c'
                nc.vector.tensor_scalar_mul(cst[:], s2[:], 0.5)  # c' state
                tcn = opool.tile([H, 2, BL, W], F32, tag="tcn")
                nc.scalar.activation(tcn[:], s2[:], AF.Tanh, scale=0.5)
                # h2 = (tanh(o/2)+1)*tanh(c), written into padded h tensor
                nc.vector.scalar_tensor_tensor(
                    hpair[:, :, :, 1:1 + W], to[:], 1.0, tcn[:],
                    ALU.add, ALU.mult)

                # conv_out -> mu (co 0), ls (co 1)
                pps = ppool.tile([H, 2, BL, W], F32, tag="ps")
                for co in range(2):
                    k = 0
                    for ci in range(2):
                        for dx in range(3):
                            nc.tensor.matmul(
                                pps[:, co],
                                band(N_IN + N_IH + N_HH + (co * 2 + ci) * 3 + dx),
                                hpair[:, ci, :, dx:dx + W],
                                start=(k == 0), stop=(k == 5),
                            )
                            k += 1

                E = opool.tile([H, BL, W], F32, tag="E")
                nc.scalar.activation(
                    E[:], pps[:, 1], AF.Exp, scale=-1.0, bias=cols[:, 1:2])
                d = opool.tile([H, BL, W], F32, tag="d")
                nc.vector.scalar_tensor_tensor(
                    d[:], pps[:, 0], cols[:, 4:5], xall[:, t + 1, :, 1:1 + W],
                    ALU.add, ALU.subtract)
                z = opool.tile([H, BL, W], F32, tag="z")
                nc.vector.tensor_mul(z[:], d[:], E[:])
                zj = opool.tile([H, BL, W], F32, tag="zjunk")
                for im in range(BL):
                    nc.scalar.activation(
                        zj[:, im, :], z[:, im, :], AF.Square,
                        accum_out=sqcols[:, im, t:t + 1])
                nc.vector.tensor_reduce(
                    lscols[:, :, t:t + 1], pps[:, 1], axis=mybir.AxisListType.X,
                    op=ALU.add)

            # ---- final reduction ----
            s_sq = opool.tile([H, BL, 1], F32, tag="ssq")
            s_ls = opool.tile([H, BL, 1], F32, tag="sls")
            nc.vector.tensor_reduce(
                s_sq[:], sqcols[:], axis=mybir.AxisListType.X, op=ALU.add)
            nc.vector.tensor_reduce(
                s_ls[:], lscols[:], axis=mybir.AxisListType.X, op=ALU.add)
            comb = opool.tile([H, BL], F32, tag="comb")
            nc.vector.scalar_tensor_tensor(
                comb[:], s_sq[:, :, 0], -0.5, s_ls[:, :, 0],
                ALU.mult, ALU.subtract)
            fps = ppool.tile([BL, 1], F32, tag="ps")
            nc.tensor.matmul(fps[:], comb[:], ones[:], start=True, stop=True)
            osb = opool.tile([BL, 1], F32, tag="osb")
            nc.vector.tensor_copy(osb[:], fps[:])
            nc.sync.dma_start(out_d[:], osb[:])

    nc.compile()
    return nc


def _get_program():
    global _CACHED
    if _CACHED is None:
        _CACHED = _build_program()
    return _CACHED


def kernel(x, Win, b_in, Wih, b_ih, Whh, b_hh, Wout, b_out):
    x = np.asarray(x, np.float32)
    Win = np.asarray(Win, np.float32)
    Wih = np.asarray(Wih, np.float32)
    Whh = np.asarray(Whh, np.float32)
    Wout = np.asarray(Wout, np.float32)
    b_in = np.asarray(b_in, np.float32)
    b_ih = np.asarray(b_ih, np.float32)
    b_hh = np.asarray(b_hh, np.float32)
    b_out = np.asarray(b_out, np.float32)

    bands = _build_bands(Win, Wih, Whh, Wout)
    bands_t = np.ascontiguousarray(
        np.transpose(bands, (1, 0, 2))).reshape(H, NBANDS * H)
    bt = bands_t.view(np.uint32)
    bt += 0x1000
    bt &= np.uint32(0xFFFFE000)

    # per-partition constant columns
    cols = np.zeros((H, 16), np.float32)
    b0, b1 = float(b_out[0]), float(b_out[1])
    cols[:, 0] = float(b_in[0])
    cols[:, 1] = -b1                       # exp bias: exp(-ls - b1)
    cols[:, 2] = np.exp(-b1)               # ch0 scale
    cols[:, 3] = -b0 * np.exp(-b1)         # ch0 bias
    cols[:, 4] = b0                        # d scalar
    gb = b_ih + b_hh                       # per-co gate bias, co=[i0,i1,g0,g1,f0,f1,o0,o1]
    for co in range(8):
        g = co // 2
        if g == 1:                         # g gate: tanh(v + b)
            cols[:, 5 + co] = gb[co]
        elif g == 2:                       # f gate: tanh((v + b + 1)/2)
            cols[:, 5 + co] = 0.5 * (gb[co] + 1.0)
        else:                              # i,o: tanh((v + b)/2)
            cols[:, 5 + co] = 0.5 * gb[co]

    # padded x planes per core: (C, H, BL*WP)
    in_maps = []
    for k in range(NCORES):
        xs = x[k * BL:(k + 1) * BL]        # (BL, C, H, W)
        xpad = np.zeros((C, H, BL, WP), np.float32)
        xpad[:, :, :, 1:1 + W] = np.transpose(xs, (1, 2, 0, 3))
        in_maps.append({
            "xp": np.ascontiguousarray(
                np.transpose(xpad, (1, 0, 2, 3))).reshape(H, C * BL * WP),
            "bands": bands_t,
            "cols": cols,
        })

    nc = _get_program()
    global _last_in_maps
    _last_in_maps = in_maps
    res = run_bass_kernel_spmd(nc, in_maps, core_ids=list(range(NCORES)))

    # assemble: add host-side constants
    const = -0.5 * LOG2PI * (H * W * C) - H * W * b1   # ch0 ls sum = H*W*b1
    out = np.zeros((B,), np.float32)
    for k in range(NCORES):
        out[k * BL:(k + 1) * BL] = res.results[k]["out"].reshape(BL) + const
    return out



# revision 11
# speedup vs baseline: 3.9141x; 3.9141x over previous
"""Trainium2 Bass kernel for nn_AutoregressiveConvLSTM.

Data-parallel over batch: 32 images -> 8 cores x 4 images.

Layout per core: every 2D field (x channel, zi channel, h) is a slab
(partition = H row, free = img*130 + 1 + w) with zero pad columns at w
offsets 0 and 129 so the 3 horizontal conv taps are free-dim offset reads.

3x3 SAME convs run on the TensorEngine as banded matmuls in fp8e4m3
DoubleRow perf mode: each instruction contracts TWO 128x128 tridiagonal
band k-tiles (vertical taps inside the band, horizontal taps as rhs
free-dim shifts) at 0.5 cycles/row. Hardware constraints (found
empirically): DR rhs access patterns are limited to [part, ktile, cols]
(3 dims) so matmuls are per-image (128 cols), and the ktile stride must
be EVEN and non-overlapping - pairs combine k-tiles from different slabs
with matching dx parity. Band weights are scaled by a power of two into
fp8 range; descales fold into downstream activation scales.

Cell math uses only tanh+exp (one ACT table): sigmoid(v)=0.5(tanh(v/2)+1),
h stored doubled (h2=2h) in fp8 slabs, 0.5 folded into the Whh/Wout bands.
State kept as cst2 = 2c. Vector path in fp16 (DVE 4x perf mode).
"""

import sys
import numpy as np

for _p in ("/opt/trn_rl_repo", "/root/.axon_site/_ro/trn_rl_repo"):
    if _p not in sys.path:
        sys.path.insert(0, _p)

import concourse.bacc as bacc
import concourse.mybir as mybir
from concourse import bass, tile
from concourse.bass_utils import run_bass_kernel_spmd

try:
    import ml_dtypes
    E4NP = (ml_dtypes.float8_e4m3fn if hasattr(ml_dtypes, "float8_e4m3fn")
            else ml_dtypes.float8_e4m3)
except ImportError:  # pragma: no cover
    E4NP = None

F32 = mybir.dt.float32
F16 = mybir.dt.float16
F8 = mybir.dt.float8e4
AF = mybir.ActivationFunctionType
ALU = mybir.AluOpType
PM = mybir.MatmulPerfMode

B, C, H, W = 32, 16, 128, 128
NCORES = 8
BL = B // NCORES          # images per core = 4
WP = W + 2                # padded row width = 130
S = BL * WP               # slab stride = 520
T = C - 1                 # recurrence steps = 15
LOG2PI = 1.8378770664093453

# rec slabs: h0=0, h1=1, zero=2, zi_t=3+t
SL_H0, SL_H1, SL_ZERO, SL_ZI = 0, 1, 2, 3
NREC = 3 + T
# x8z slabs: zero=0, x_c=1+c
NX8 = 1 + C

# band pair table: pair p -> lhsT [H, 2, H]
P_IN = 0                   # +d: (0, Bin_d)
P_G = 3                    # +co*4+q: (h00,h10) (h01,h11) (h02,zi0) (h12,zi2)
P_T0 = 35                  # +co*3+d: (0, Bzi_d); q4 of step t reuses d=1
P_O = 59                   # +oc*3+d: (o_h0d, o_h1d)
NPAIR = 65


def _band(w_col):
    """128x128 tridiagonal lhsT for one ky tap column: lhsT[h_in, h_out]."""
    Bm = np.zeros((H, H), np.float32)
    idx = np.arange(H)
    for ky in range(3):
        hh = idx + ky - 1
        m = (hh >= 0) & (hh < H)
        Bm[hh[m], idx[m]] = w_col[ky]
    return Bm


def _pow2_scale(maxv, target=192.0):
    if maxv <= 0:
        return 1.0
    return float(2.0 ** np.floor(np.log2(target / maxv)))


def _build_bands(Win, Wih, Whh, Wout, S_in, S_g, S_out):
    """All DoubleRow lhsT pairs as one (NPAIR, 2, H, H) fp32 array (scaled)."""
    pairs = np.zeros((NPAIR, 2, H, H), np.float32)

    def bin_(dx):
        return S_in * _band(Win[:, dx, 0, 0])

    def bzi(co, dx):
        # zi slabs hold S_in*zi, so descale S_in here
        return (S_g / S_in) * _band(Wih[:, dx, 0, co])

    def bh(co, ci, dx):
        return 0.5 * S_g * _band(Whh[:, dx, ci, co])

    def bo(oc, ci, dx):
        return 0.5 * S_out * _band(Wout[:, dx, ci, oc])

    for d in range(3):
        pairs[P_IN + d, 1] = bin_(d)
    for co in range(8):
        g = P_G + co * 4
        pairs[g + 0, 0], pairs[g + 0, 1] = bh(co, 0, 0), bh(co, 1, 0)
        pairs[g + 1, 0], pairs[g + 1, 1] = bh(co, 0, 1), bh(co, 1, 1)
        pairs[g + 2, 0], pairs[g + 2, 1] = bh(co, 0, 2), bzi(co, 0)
        pairs[g + 3, 0], pairs[g + 3, 1] = bh(co, 1, 2), bzi(co, 2)
        for d in range(3):
            pairs[P_T0 + co * 3 + d, 1] = bzi(co, d)
    for oc in range(2):
        for d in range(3):
            pairs[P_O + oc * 3 + d, 0] = bo(oc, 0, d)
            pairs[P_O + oc * 3 + d, 1] = bo(oc, 1, d)
    return pairs


def _ktile_pair(ap, delta):
    """Insert the DoubleRow k-tile dim [delta, 2] after the partition dim."""
    assert delta % 2 == 0 and delta >= W
    ap2 = ap.copy()
    ap2.ap = ap.ap[:1] + [[delta, 2]] + [list(d) for d in ap.ap[1:]]
    return ap2


_CACHED = {}


def _build_program(scal):
    """scal: dict of python-float scalar params baked into the program."""
    nc = bacc.Bacc(None, target_bir_lowering=False)

    xp_d = nc.dram_tensor("xp", [H, C * S], F32, kind="ExternalInput")
    x8_d = nc.dram_tensor("x8", [H, NX8 * S], F8, kind="ExternalInput")
    bands_d = nc.dram_tensor("bands", [H, NPAIR * 2 * H], F8,
                             kind="ExternalInput")
    cols_d = nc.dram_tensor("cols", [H, 16], F32, kind="ExternalInput")
    out_d = nc.dram_tensor("out", [8, 1], F32, kind="ExternalOutput")

    inv_Sin = 1.0 / scal["S_in"]
    inv_Sg = 1.0 / scal["S_g"]
    inv_So = 1.0 / scal["S_out"]
    b_in = scal["b_in"]
    b0, b1 = scal["b0"], scal["b1"]
    gbias = scal["gbias"]          # tuple of 8 gate biases (tanh form)
    gscale = (0.5 * inv_Sg, 0.5 * inv_Sg, inv_Sg, inv_Sg,
              0.5 * inv_Sg, 0.5 * inv_Sg, 0.5 * inv_Sg, 0.5 * inv_Sg)

    with tile.TileContext(nc) as tc:
        with (
            tc.tile_pool(name="const", bufs=1) as cpool,
            tc.tile_pool(name="state", bufs=1) as spool,
            tc.tile_pool(name="psA", bufs=3, space=bass.MemorySpace.PSUM) as ppool,
            tc.tile_pool(name="psB", bufs=1, space=bass.MemorySpace.PSUM) as zpool,
            tc.tile_pool(name="psC", bufs=1, space=bass.MemorySpace.PSUM) as fpool,
        ):
            xall = cpool.tile([H, C, BL, WP], F32, tag="xall")
            x8z = cpool.tile([H, NX8, BL, WP], F8, tag="x8z")
            bands = cpool.tile([H, NPAIR, 2, H], F8, tag="bands")
            rec = cpool.tile([H, NREC, BL, WP], F8, tag="rec")
            ones = cpool.tile([H, 1], F32, tag="ones")
            cols = cpool.tile([H, 16], F32, tag="cols")

            sqcols = spool.tile([H, BL, 16], F32, tag="sqcols")
            lscols = spool.tile([H, BL, 16], F32, tag="lscols")
            cst2 = spool.tile([H, 2, BL, W], F16, tag="cst2")
            tf = spool.tile([H, 2, BL, W], F16, tag="tf")
            ti = spool.tile([H, 2, BL, W], F16, tag="ti")
            tg = spool.tile([H, 2, BL, W], F16, tag="tg")
            to = spool.tile([H, 2, BL, W], F16, tag="to")
            u1 = spool.tile([H, 2, BL, W], F16, tag="u1")
            u2 = spool.tile([H, 2, BL, W], F16, tag="u2")
            tcn = spool.tile([H, 2, BL, W], F16, tag="tcn")
            Et = spool.tile([H, BL, W], F16, tag="Et")
            t1 = spool.tile([H, BL, W], F16, tag="t1")
            zt = spool.tile([H, BL, W], F16, tag="zt")
            z0 = spool.tile([H, BL, W], F16, tag="z0")
            zsq = spool.tile([H, BL, W], F16, tag="zsq")
            comb = spool.tile([H, 2, BL, 1], F32, tag="comb")

            # ---- loads ----
            nc.sync.dma_start(x8z[:], x8_d[:])
            nc.sync.dma_start(bands[:], bands_d[:])
            nc.sync.dma_start(cols[:], cols_d[:])
            nc.sync.dma_start(xall[:, 0:2], xp_d[:, 0:2 * S])
            nc.sync.dma_start(xall[:, 2:C], xp_d[:, 2 * S:])

            nc.gpsimd.memset(rec[:], 0.0)
            nc.gpsimd.memset(sqcols[:], 0.0)
            nc.gpsimd.memset(lscols[:], 0.0)
            nc.gpsimd.memset(ones[:], 1.0)

            def pair_mm(out_ap, p, base_ap, delta, start, stop):
                nc.tensor.matmul(out_ap, bands[:, p],
                                 _ktile_pair(base_ap, delta),
                                 start=start, stop=stop,
                                 perf_mode=PM.DoubleRow,
                                 skip_group_check=True)

            def conv_in(t):
                """zi_t = fp8(S_in*(conv_in(x8_t) + b_in)) into rec slab;
                the S_in descale is folded into the gate zi-bands."""
                zps = zpool.tile([H, BL, W], F32, tag="zps")
                for b in range(BL):
                    for d in range(3):
                        pair_mm(zps[:, b], P_IN + d, x8z[:, 0, b, d:d + W],
                                (1 + t) * S, d == 0, d == 2)
                dst = rec[:, SL_ZI + t, :, 1:1 + W]
                if b_in == 0.0:
                    nc.vector.tensor_copy(dst, zps[:])
                else:
                    nc.vector.tensor_scalar(
                        dst, zps[:], 1.0, scal["S_in"] * b_in,
                        ALU.mult, ALU.add)

            def gate_zi_start(t, co_pair, gps):
                """h-independent (zero, zi1) group starters for one gate."""
                for co in co_pair:
                    for b in range(BL):
                        pair_mm(gps[:, co % 2, b], P_T0 + co * 3 + 1,
                                rec[:, SL_ZERO, b, 1:1 + W], (1 + t) * S,
                                True, False)

            def gate_finish(t, co, gps):
                """h-dependent k-tile pairs q0..q3 for one output channel."""
                g = P_G + co * 4
                for b in range(BL):
                    out_ap = gps[:, co % 2, b]
                    pair_mm(out_ap, g + 0, rec[:, SL_H0, b, 0:W], S,
                            False, False)
                    pair_mm(out_ap, g + 1, rec[:, SL_H0, b, 1:1 + W], S,
                            False, False)
                    pair_mm(out_ap, g + 2, rec[:, SL_H0, b, 2:2 + W],
                            (3 + t) * S - 2, False, False)
                    pair_mm(out_ap, g + 3, rec[:, SL_H1, b, 2:2 + W],
                            (2 + t) * S, False, True)

            def gate_t0(co, gps):
                """t=0: zi taps only."""
                for b in range(BL):
                    out_ap = gps[:, co % 2, b]
                    for d in range(3):
                        pair_mm(out_ap, P_T0 + co * 3 + d,
                                rec[:, SL_ZERO, b, d:d + W], S,
                                d == 0, d == 2)

            def gate_tanh(gidx, gps, dst, half=None):
                c0 = 2 * gidx
                if half is None and gbias[c0] == gbias[c0 + 1]:
                    nc.scalar.activation(dst[:], gps[:], AF.Tanh,
                                         scale=gscale[c0],
                                         bias=cols[:, c0:c0 + 1])
                else:
                    halves = range(2) if half is None else (half,)
                    for hh in halves:
                        nc.scalar.activation(dst[:, hh], gps[:, hh],
                                             AF.Tanh, scale=gscale[c0 + hh],
                                             bias=cols[:, c0 + hh:c0 + hh + 1])

            def out_conv(pps):
                """conv_out on current h slabs -> pps[:, oc] (scaled S_out)."""
                for oc in range(2):
                    for b in range(BL):
                        for d in range(3):
                            pair_mm(pps[:, oc, b], P_O + oc * 3 + d,
                                    rec[:, SL_H0, b, d:d + W], S,
                                    d == 0, d == 2)

            def z_path(pps, ch):
                """log-prob pieces for channel ch from pps (mu,ls scaled)."""
                nc.scalar.activation(Et[:], pps[:, 1], AF.Exp,
                                     scale=-inv_So, bias=cols[:, 8:9])
                nc.vector.scalar_tensor_tensor(
                    t1[:], pps[:, 0], inv_So, xall[:, ch, :, 1:1 + W],
                    ALU.mult, ALU.subtract)
                if b0 == 0.0:
                    nc.gpsimd.tensor_tensor(zt[:], t1[:], Et[:], ALU.mult)
                    nc.gpsimd.tensor_tensor(zsq[:], zt[:], zt[:], ALU.mult)
                else:
                    nc.vector.scalar_tensor_tensor(
                        zt[:], t1[:], b0, Et[:], ALU.add, ALU.mult)
                    nc.vector.scalar_tensor_tensor(
                        zsq[:], zt[:], 1.0, zt[:], ALU.mult, ALU.mult)
                nc.vector.tensor_reduce(
                    sqcols[:, :, ch:ch + 1], zsq[:],
                    axis=mybir.AxisListType.X, op=ALU.add)
                nc.vector.tensor_reduce(
                    lscols[:, :, ch - 1:ch], pps[:, 1],
                    axis=mybir.AxisListType.X, op=ALU.add)

            # ---- preamble: zi_0, zi_1; channel-0 log prob ----
            conv_in(0)
            conv_in(1)
            nc.vector.tensor_scalar(
                z0[:], xall[:, 0, :, 1:1 + W], b0, float(np.exp(-b1)),
                ALU.subtract, ALU.mult)
            nc.vector.scalar_tensor_tensor(
                zsq[:], z0[:], 1.0, z0[:], ALU.mult, ALU.mult)
            nc.vector.tensor_reduce(
                sqcols[:, :, 0:1], zsq[:], axis=mybir.AxisListType.X,
                op=ALU.add)

            # ---- recurrence ----
            # gate tiles f,i allocated in the previous step (their zi-start
            # matmuls are PE filler during the h8 wait); g,o allocate after
            # tanh-f/tanh-i free pool slots. out-conv of step t runs right
            # after h8(t); the z-path consumes it early in step t+1.
            gate_dst = (tf, ti, tg, to)
            gps_f = gps_i = None
            for t in range(T):
                if t == 0:
                    gps_f = ppool.tile([H, 2, BL, W], F32, tag="ps")
                    gps_i = ppool.tile([H, 2, BL, W], F32, tag="ps")
                    conv_in(2)
                # f, i gates
                for g in range(2):
                    gps = (gps_f, gps_i)[g]
                    for co in (2 * g, 2 * g + 1):
                        if t == 0:
                            gate_t0(co, gps)
                        else:
                            gate_finish(t, co, gps)
                    gate_tanh(g, gps, gate_dst[g])
                    if g == 0 and t >= 1:
                        nc.vector.scalar_tensor_tensor(
                            u1[:], tf[:], 1.0, cst2[:], ALU.add, ALU.mult)
                # g gate: per-half tanh, then per-half c-update chain
                gps_g = ppool.tile([H, 2, BL, W], F32, tag="ps")
                if t == 0:
                    for co in (4, 5):
                        gate_t0(co, gps_g)
                        gate_tanh(2, gps_g, tg, half=co % 2)
                else:
                    gate_zi_start(t, (4, 5), gps_g)
                    for co in (4, 5):
                        gate_finish(t, co, gps_g)
                        gate_tanh(2, gps_g, tg, half=co % 2)
                for hh in range(2):
                    nc.vector.scalar_tensor_tensor(
                        u2[:, hh], ti[:, hh], 1.0, tg[:, hh],
                        ALU.add, ALU.mult)
                    if t == 0:
                        nc.vector.tensor_copy(cst2[:, hh], u2[:, hh])
                    else:
                        nc.vector.scalar_tensor_tensor(
                            cst2[:, hh], u1[:, hh], 0.5, u2[:, hh],
                            ALU.mult, ALU.add)
                    nc.scalar.activation(tcn[:, hh], cst2[:, hh], AF.Tanh,
                                         scale=0.5)
                # o gate: per-half tanh, h8 right after each half
                gps_o = ppool.tile([H, 2, BL, W], F32, tag="ps")
                if t == 0:
                    for co in (6, 7):
                        gate_t0(co, gps_o)
                        gate_tanh(3, gps_o, to, half=co % 2)
                else:
                    gate_zi_start(t, (6, 7), gps_o)
                    for co in (6, 7):
                        gate_finish(t, co, gps_o)
                        gate_tanh(3, gps_o, to, half=co % 2)
                # next step's h-independent PE filler before h8 blocks PE
                if t + 1 < T:
                    gps_f = ppool.tile([H, 2, BL, W], F32, tag="ps")
                    gps_i = ppool.tile([H, 2, BL, W], F32, tag="ps")
                    gate_zi_start(t + 1, (0, 1), gps_f)
                    gate_zi_start(t + 1, (2, 3), gps_i)
                    if t + 3 < T:
                        conv_in(t + 3)
                # h2 = (to+1)*tanh(c) -> fp8 h slabs, per half
                for ci in range(2):
                    nc.vector.scalar_tensor_tensor(
                        rec[:, SL_H0 + ci, :, 1:1 + W], to[:, ci], 1.0,
                        tcn[:, ci], ALU.add, ALU.mult)
                # out-conv of this step's h -> params for channel t+1
                pps = ppool.tile([H, 2, BL, W], F32, tag="ps")
                out_conv(pps)
                z_path(pps, t + 1)

            # ---- final reduction ----
            nc.vector.tensor_reduce(
                comb[:, 0], sqcols[:], axis=mybir.AxisListType.X, op=ALU.add)
            nc.vector.tensor_reduce(
                comb[:, 1], lscols[:], axis=mybir.AxisListType.X, op=ALU.add)
            fps = fpool.tile([8, 1], F32, tag="fps")
            nc.tensor.matmul(fps[:], comb[:], ones[:], start=True, stop=True)
            osb = spool.tile([8, 1], F32, tag="osb")
            nc.vector.tensor_copy(osb[:], fps[:])
            nc.sync.dma_start(out_d[:], osb[:])

    nc.compile()
    return nc


def _get_program(scal):
    key = tuple(sorted(scal.items()))
    if key not in _CACHED:
        _CACHED[key] = _build_program(scal)
    return _CACHED[key]


def kernel(x, Win, b_in, Wih, b_ih, Whh, b_hh, Wout, b_out):
    x = np.asarray(x, np.float32)
    Win = np.asarray(Win, np.float32)
    Wih = np.asarray(Wih, np.float32)
    Whh = np.asarray(Whh, np.float32)
    Wout = np.asarray(Wout, np.float32)
    b_in_v = float(np.asarray(b_in, np.float32)[0])
    b_ih_v = np.asarray(b_ih, np.float32)
    b_hh_v = np.asarray(b_hh, np.float32)
    b0, b1 = [float(v) for v in np.asarray(b_out, np.float32)]

    # zi8 = S_in*zi is stored directly; bound |zi| by 9*max|Win|*max|x|
    zi_bound = 9.0 * float(np.abs(Win).max()) * 5.5 + abs(b_in_v)
    S_in = _pow2_scale(zi_bound, target=384.0)
    S_g = _pow2_scale(max(float(np.abs(Wih).max()),
                          0.5 * float(np.abs(Whh).max())))
    S_out = _pow2_scale(0.5 * float(np.abs(Wout[:, :, :2, :]).max()))

    gb = b_ih_v + b_hh_v
    gbias = []
    for co in range(8):
        g = co // 2
        if g == 1:
            gbias.append(float(gb[co]))
        elif g == 2:
            gbias.append(float(0.5 * (gb[co] + 1.0)))
        else:
            gbias.append(float(0.5 * gb[co]))

    scal = dict(S_in=S_in, S_g=S_g, S_out=S_out, b_in=b_in_v, b0=b0, b1=b1,
                gbias=tuple(gbias))

    cols_host = np.zeros((H, 16), np.float32)
    for co in range(8):
        cols_host[:, co] = gbias[co]
    cols_host[:, 8] = -b1

    pairs = _build_bands(Win, Wih, Whh, Wout, S_in, S_g, S_out)
    # device layout [K=H(part), NPAIR, 2, M=H]
    bands_dev = np.ascontiguousarray(
        np.transpose(pairs, (2, 0, 1, 3))).astype(E4NP)
    bands_flat = bands_dev.reshape(H, NPAIR * 2 * H)

    in_maps = []
    for k in range(NCORES):
        xs = x[k * BL:(k + 1) * BL]            # (BL, C, H, W)
        xpad = np.zeros((C, H, BL, WP), np.float32)
        xpad[:, :, :, 1:1 + W] = np.transpose(xs, (1, 2, 0, 3))
        xp = np.ascontiguousarray(
            np.transpose(xpad, (1, 0, 2, 3))).reshape(H, C * S)
        x8 = np.zeros((H, NX8, BL, WP), E4NP)
        x8[:, 1:] = np.transpose(xpad, (1, 0, 2, 3)).astype(E4NP)
        in_maps.append({
            "xp": xp,
            "x8": x8.reshape(H, NX8 * S),
            "bands": bands_flat,
            "cols": cols_host,
        })

    nc = _get_program(scal)
    global _last_in_maps, _last_scal
    _last_in_maps = in_maps
    _last_scal = scal
    res = run_bass_kernel_spmd(nc, in_maps, core_ids=list(range(NCORES)))

    const = -0.5 * LOG2PI * (H * W * C) - H * W * b1
    out = np.zeros((B,), np.float32)
    for k in range(NCORES):
        r = res.results[k]["out"].reshape(8)
        out[k * BL:(k + 1) * BL] = -0.5 * r[0:4] - r[4:8] / S_out + const
    return out
